# revision 1
# baseline (speedup 1.0000x reference)
"""Distributed Bass attention kernel for 8 TRN2 NeuronCores.

Problem: full-dim attention (no head split), x:(2,4096,2048), 4x 2048^2 weights.

Sharding: batch+sequence parallel. Core c owns batch b=c//4 and query rows
[1024*(c%4), 1024*(c%4+1)). Each core computes its local K^T/V shard; the
shards are AllGather-ed within the 4-core replica group of the same batch
(K right after the k-projection, V after the q/v-projections, both hidden
behind the remaining projections + the score phase). Then each core computes
transposed scores+softmax for its 8 q-tiles (sharing each streamed K chunk),
ctx^T (sharing each streamed V column chunk), and the output projection.
Host reassembles rows.

Perf notes baked in:
- All TensorE math bf16 (fp32 PSUM accumulate); rel err ~5e-3.
- Scores are computed TRANSPOSED (lhsT=K-tile, rhs=q^T) so exp writes P^T
  directly -- no PE transposes, no PSUM->SBUF P copies.
- Softmax needs no max subtraction (scores ~ N(0,1)). Row sums come from a
  ones-matmul over P^T (l replicated across partitions); 1/l is applied by
  the DVE during the ctx^T PSUM->SBUF copy.
- Weights/x arrive host-pre-tiled so weight DMAs are contiguous [128, N]
  blocks: HWDGE descriptor-gen (2-4us per strided DMA, strictly in-order on
  the issuing engine) was a serial bottleneck.
- DMA issues are split across both HWDGE rings: sync (x, bounce, V streams,
  outputs) and scalar (weights, K streams). Projection PSUM evacuation runs
  on the vector engine so the scalar engine's queue stays short.
- Small stores are staged and merged (v rows per quarter, out rows per
  f-quarter) to cut DMA-issue count.
"""

import numpy as np
import ml_dtypes

BF16 = ml_dtypes.bfloat16

D = 2048          # model dim
S = 4096          # sequence length per batch
BATCH = 2
NCORES = 8
GROUP = 4         # replica group size (cores per batch)
ROWS = S // GROUP  # query rows per core = 1024
P = 128           # partitions
DT = D // P       # 16 d-tiles
IT = ROWS // P    # 8 i-tiles per core
JT = S // P       # 32 j-tiles (full seq)
NCH = S // 512    # 8 key chunks
SCALE = 1.0 / float(np.sqrt(D))

_CACHE = {}


def _build():
    from concourse import bacc, mybir, tile

    f32 = mybir.dt.float32
    bf16 = mybir.dt.bfloat16

    nc = bacc.Bacc("TRN2", target_bir_lowering=False, debug=False,
                   num_devices=NCORES)

    # host-pre-tiled inputs (see _in_maps): every load is contiguous rows
    xt_d = nc.dram_tensor("xt", [P, DT * ROWS], bf16, kind="ExternalInput")
    wqt_d = nc.dram_tensor("wqt", [DT, P, DT * P], bf16, kind="ExternalInput")
    wkt_d = nc.dram_tensor("wkt", [DT, P, DT * P], bf16, kind="ExternalInput")
    wvt_d = nc.dram_tensor("wvt", [4, P, DT * 512], bf16, kind="ExternalInput")
    wot_d = nc.dram_tensor("wot", [4, P, DT * 512], bf16, kind="ExternalInput")
    out_d = nc.dram_tensor("out", [ROWS, D], f32, kind="ExternalOutput")

    RG = [[0, 1, 2, 3], [4, 5, 6, 7]]

    def all_gather(src, dst):
        return nc.gpsimd.collective_compute(
            "AllGather", mybir.AluOpType.bypass, replica_groups=RG,
            ins=[src.opt()], outs=[dst.opt()])

    with tile.TileContext(nc) as tc:
        with (
            tc.tile_pool(name="dram", bufs=1, space="DRAM") as dram,
            tc.tile_pool(name="persist", bufs=1) as persist,
            tc.tile_pool(name="psum", bufs=2, space="PSUM") as psum,
        ):
            kt_b = dram.tile([D, ROWS], bf16)
            v_b = dram.tile([ROWS, D], bf16)
            kt_g = dram.tile([GROUP, D, ROWS], bf16)
            v_g = dram.tile([GROUP, ROWS, D], bf16)

            linv_bc = persist.tile([P, ROWS], f32)  # 1/l bcast on partitions
            ones = persist.tile([P, P], bf16)

            with tc.tile_pool(name="qtpool", bufs=1) as qtpool:
                # q^T [e, i] during proj+scores; ctx^T [d', i] afterwards
                qt_s = qtpool.tile([P, DT, ROWS], bf16)
                ctxt_s = qt_s

                # ---------------- Phase 1: projections ----------------
                with tc.tile_pool(name="proj", bufs=2) as proj:
                    # warm both HWDGE rings so the first real loads skip
                    # the first-DMA spin-up latency
                    warm = proj.tile([P, 16], bf16, bufs=1)
                    nc.sync.dma_start(out=warm[0:1, :], in_=xt_d[0:1, 0:16])
                    nc.scalar.dma_start(out=warm[1:2, :], in_=xt_d[1:2, 0:16])
                    xt_s = proj.tile([P, DT, ROWS], bf16, bufs=1)
                    xt_v = xt_d[:].rearrange("p (t i) -> p t i", t=DT)
                    for c in range(2):
                        eng = nc.sync if c == 0 else nc.scalar
                        eng.dma_start(
                            out=xt_s[:, :, c * 512:(c + 1) * 512],
                            in_=xt_v[:, :, c * 512:(c + 1) * 512])

                    def kq_proj(w_d, is_k, pre=()):
                        # out[e-tile, i-chunk] = sum_d wt[d,e]^T x^T[d,i]
                        for et in range(DT):
                            if et < len(pre):
                                wcol = pre[et]
                            else:
                                wcol = proj.tile([P, DT, P], bf16,
                                                 tag="wcol", bufs=6)
                                nc.scalar.dma_start(out=wcol[:], in_=w_d[et])
                            kt_t = proj.tile([P, ROWS], bf16, tag="kt_t",
                                             bufs=3)
                            for c in range(2):
                                ps = psum.tile([P, 512], f32, tag="acc")
                                for dt_i in range(DT):
                                    nc.tensor.matmul(
                                        ps[:],
                                        wcol[:, dt_i, :],
                                        xt_s[:, dt_i, c * 512:(c + 1) * 512],
                                        start=(dt_i == 0),
                                        stop=(dt_i == DT - 1))
                                dst = (kt_t[:, c * 512:(c + 1) * 512]
                                       if is_k else
                                       qt_s[:, et, c * 512:(c + 1) * 512])
                                nc.vector.tensor_copy(dst, ps[:])
                            if is_k:
                                nc.sync.dma_start(
                                    out=kt_b[et * P:(et + 1) * P, :],
                                    in_=kt_t[:])

                    def v_proj():
                        # v: out[j-tile, d'] = sum_d x^T[d,j]^T wvt[d,d']
                        for qd in range(4):
                            wvq = proj.tile([P, DT, 512], bf16, tag="wq4",
                                            bufs=4)
                            nc.scalar.dma_start(out=wvq[:], in_=wvt_d[qd])
                            v_stage = proj.tile([P, IT, 512], bf16,
                                                tag="v_stage", bufs=2)
                            for jt in range(IT):
                                ps = psum.tile([P, 512], f32, tag="acc")
                                for dt_i in range(DT):
                                    nc.tensor.matmul(
                                        ps[:],
                                        xt_s[:, dt_i, jt * P:(jt + 1) * P],
                                        wvq[:, dt_i, :],
                                        start=(dt_i == 0),
                                        stop=(dt_i == DT - 1))
                                nc.vector.tensor_copy(v_stage[:, jt, :],
                                                      ps[:])
                            nc.sync.dma_start(
                                out=v_b[:, qd * 512:(qd + 1) * 512]
                                .rearrange("(jt p) d -> p jt d", p=P),
                                in_=v_stage[:])

                    kq_proj(wkt_d, True)
                    all_gather(kt_b, kt_g)
                    pre_q = []
                    for et in range(2):
                        wcol = proj.tile([P, DT, P], bf16, tag="wcol",
                                         bufs=6, name=f"wcol_pre{et}")
                        nc.scalar.dma_start(out=wcol[:], in_=wqt_d[et])
                        pre_q.append(wcol)
                    v_proj()
                    cc_v = all_gather(v_b, v_g)
                    kq_proj(wqt_d, False, pre=pre_q)

                # ------------- Phase 2: attention -------------
                with tc.tile_pool(name="cpool", bufs=1) as cpool:
                    with tc.tile_pool(name="attn", bufs=2) as attn:
                        pt_s = attn.tile([P, JT, IT * P], bf16, bufs=1)
                        # --- A: transposed scores + exp, all 8 i-tiles ---
                        for cidx in range(2 * NCH):  # 16 chunks of 256 keys
                            r, q4 = cidx // GROUP, cidx % GROUP
                            kbuf = attn.tile([P, DT, 256], bf16,
                                             tag="kbuf", bufs=6)
                            # alternate rings: two parallel HWDGE FIFOs keep
                            # the K stream ahead of compute while AG(V)'s
                            # SDMA traffic slows individual transfers
                            eng = nc.sync if cidx % 2 == 0 else nc.scalar
                            kdma = eng.dma_start(
                                out=kbuf[:],
                                in_=kt_g[r, :, q4 * 256:(q4 + 1) * 256]
                                .rearrange("(t p) j -> p t j", p=P))
                            if cidx == 8:
                                # Hold AG(V) until most of the K stream has
                                # landed: without this the K chunk loads
                                # crawl at the AG-contended DMA rate and
                                # stall the score phase; with it they burst
                                # at full HBM rate and AG(V) still finishes
                                # before the ctx phase needs it.
                                from concourse.bass import _add_dep_helper
                                _add_dep_helper(
                                    cc_v.ins, kdma.ins, sync=True,
                                    reason="delay AG(V) past K stream")
                            for jl in range(2):
                                jt = cidx * 2 + jl
                                for ib in range(2):
                                    sps = psum.tile([P, 512], f32,
                                                    tag="scores", bufs=3)
                                    for e in range(DT):
                                        nc.tensor.matmul(
                                            sps[:],
                                            kbuf[:, e, jl * P:(jl + 1) * P],
                                            qt_s[:, e, ib * 512:
                                                 (ib + 1) * 512],
                                            start=(e == 0),
                                            stop=(e == DT - 1))
                                    nc.scalar.activation(
                                        pt_s[:, jt, ib * 512:(ib + 1) * 512],
                                        sps[:],
                                        mybir.ActivationFunctionType.Exp,
                                        scale=SCALE)
                        # --- rowsums via ones-matmul: l bcast on partitions
                        nc.gpsimd.memset(ones[:], 1.0)
                        for ib in range(2):
                            lps = psum.tile([P, 512], f32, tag="ctx")
                            for jt in range(JT):
                                nc.tensor.matmul(
                                    lps[:], ones[:],
                                    pt_s[:, jt, ib * 512:(ib + 1) * 512],
                                    start=(jt == 0), stop=(jt == JT - 1))
                            nc.vector.reciprocal(
                                linv_bc[:, ib * 512:(ib + 1) * 512], lps[:])
                        # --- B: ctx^T[d', i] = sum_j V[j,d']^T P^T[j, i],
                        #     scaled by 1/l during PSUM evacuation ---
                        for dp2 in range(DT // 2):  # pairs of d'-tiles
                            vcol = attn.tile([P, NCH, 4, 256], bf16,
                                             tag="vcol", bufs=2)
                            for g in range(NCH):  # j-block [512g, 512g+512)
                                r, h = g // 2, g % 2
                                nc.sync.dma_start(
                                    out=vcol[:, g, :, :],
                                    in_=v_g[r, h * 512:(h + 1) * 512,
                                            dp2 * 256:(dp2 + 1) * 256]
                                    .rearrange("(t p) d -> p t d", p=P))
                            for ds in range(2):
                                dp = dp2 * 2 + ds
                                for ih in range(2):  # i-halves of 512
                                    cps = psum.tile([P, 512], f32, tag="ctx")
                                    for jt in range(JT):
                                        nc.tensor.matmul(
                                            cps[:],
                                            vcol[:, jt // 4, jt % 4,
                                                 ds * P:(ds + 1) * P],
                                            pt_s[:, jt, ih * 512:
                                                 (ih + 1) * 512],
                                            start=(jt == 0),
                                            stop=(jt == JT - 1))
                                    nc.vector.tensor_tensor(
                                        out=ctxt_s[:, dp, ih * 512:
                                                   (ih + 1) * 512],
                                        in0=cps[:],
                                        in1=linv_bc[:, ih * 512:
                                                    (ih + 1) * 512],
                                        op=mybir.AluOpType.mult)

                    # ------------- Phase 3: output projection -------------
                    with tc.tile_pool(name="oproj", bufs=2) as oproj:
                        for fq in range(4):
                            woq = oproj.tile([P, DT, 512], bf16,
                                             tag="woq", bufs=2)
                            nc.scalar.dma_start(out=woq[:], in_=wot_d[fq])
                            osb = oproj.tile([P, IT, 512], f32, tag="osb",
                                             bufs=2)
                            for it in range(IT):
                                ops = psum.tile([P, 512], f32, tag="acc")
                                for dp in range(DT):
                                    nc.tensor.matmul(
                                        ops[:],
                                        ctxt_s[:, dp, it * P:(it + 1) * P],
                                        woq[:, dp, :],
                                        start=(dp == 0), stop=(dp == DT - 1))
                                nc.scalar.copy(osb[:, it, :], ops[:])
                            for oh in ((0, 1) if fq == 3 else (None,)):
                                if oh is None:
                                    lo, hi = 0, IT
                                else:
                                    lo, hi = oh * 4, oh * 4 + 4
                                nc.sync.dma_start(
                                    out=out_d[lo * P:hi * P,
                                              fq * 512:(fq + 1) * 512]
                                    .rearrange("(it p) f -> p it f", p=P),
                                    in_=osb[:, lo:hi, :])

    nc.compile()
    return nc


def _get_nc():
    if "nc" not in _CACHE:
        _CACHE["nc"] = _build()
    return _CACHE["nc"]


def _tile_we(w):
    # [out,in] weight -> w.T tiled as [et, p, dt*128] contiguous
    wt = np.ascontiguousarray(np.asarray(w, np.float32).T)  # [d, e]
    t = wt.reshape(DT, P, DT, P).transpose(2, 1, 0, 3)      # [et, p, dt, e]
    return np.ascontiguousarray(t.reshape(DT, P, DT * P)).astype(BF16)


def _tile_wq4(w):
    # [out,in] weight -> w.T tiled as [qd, p, dt*512] contiguous
    wt = np.ascontiguousarray(np.asarray(w, np.float32).T)  # [d, dcol]
    t = wt.reshape(DT, P, 4, 512).transpose(2, 1, 0, 3)     # [qd, p, dt, dc]
    return np.ascontiguousarray(t.reshape(4, P, DT * 512)).astype(BF16)


def _in_maps(x, wq, wk, wv, wo):
    wqt = _tile_we(wq)
    wkt = _tile_we(wk)
    wvt = _tile_wq4(wv)
    wot = _tile_wq4(wo)
    x = np.asarray(x, np.float32)
    maps = []
    for c in range(NCORES):
        b, r = c // GROUP, c % GROUP
        xt = x[b, r * ROWS:(r + 1) * ROWS, :].T          # [d, i]
        xt = xt.reshape(DT, P, ROWS).transpose(1, 0, 2)  # [p, dt, i]
        xt = np.ascontiguousarray(xt.reshape(P, DT * ROWS)).astype(BF16)
        maps.append({"xt": xt, "wqt": wqt, "wkt": wkt, "wvt": wvt,
                     "wot": wot})
    return maps


def run(x, wq, wk, wv, wo, trace=False, **trace_kwargs):
    from concourse.bass_utils import run_bass_kernel_spmd
    nc = _get_nc()
    res = run_bass_kernel_spmd(nc, _in_maps(x, wq, wk, wv, wo),
                               list(range(NCORES)), trace=trace,
                               **trace_kwargs)
    out = np.empty((BATCH, S, D), np.float32)
    for c in range(NCORES):
        b, r = c // GROUP, c % GROUP
        out[b, r * ROWS:(r + 1) * ROWS, :] = res.results[c]["out"]
    return out, res


def kernel(x, wq, wk, wv, wo):
    out, _ = run(x, wq, wk, wv, wo)
    return out

